# revision 29
# baseline (speedup 1.0000x reference)
"""NeuralTPP log-likelihood kernel for 8x Trainium2 NeuronCores.

Reference computation (per batch row b):
  t = max(times, 1e-8); logt = log(t); x = [t, logt]
  h_s = tanh(W_ih x_s + b_ih + b_hh + W_hh h_{s-1}),  h_{-1} = 0   (S=2048)
  [mu_s, logsig_s] = W_lin h_{s-1} + b_lin
  z_s = (logt_s - mu_s) / exp(logsig_s)
  log_density = sum_{s<=S-2} mask[s+1] * (-logt_s - logsig_s - C - z_s^2/2)
  last = log(0.5 - 0.5*erf(z_{s*}/sqrt(2))),  s* = sum(mask) - 1
  out  = log_density + last

Strategy: data parallel over batch (32 rows/core) PLUS sequence-parallel
within each core. The tanh RNN is strongly contractive (cold restart
converges to float noise in <48 steps), so S=2048 splits into 16 chunks of
128 steps, each warmed up from h=0 over the preceding 32 steps. All chunks
advance in lockstep: the 2048-step serial scan becomes 160 steps of
512-wide ops (col = 32*chunk + b), run as two 256-wide half-chains (A =
chunks 0-7, B = 8-15) so PE-matmul and ACT-tanh of the two halves overlap.
x-projections are pre-accumulated into PSUM 2 steps per bank; the output
side (mu/sigma/log-prob) runs on ring windows of 16 steps, one window
behind the scan, via a PE-op fifo drained between chain matmuls.
"""
import heapq
import itertools
import numpy as np
from contextlib import ExitStack

import concourse.bacc as bacc
import concourse.bass as bass
import concourse.tile as tile
import concourse.mybir as mybir
from concourse import bass2jax

B, S, H = 256, 2048, 128
NCORES = 8
BL = B // NCORES          # 32 batch rows per core
P = 16                    # sequence chunks
CH = S // P               # 128 steps per chunk
WU = 32                   # warmup steps
NSTEP = WU + CH           # 160 serial steps
NWIN = NSTEP // 16        # 10 ring windows (2 warmup + 8 real)
NRW = CH // 16            # 8 real windows
HWD = 256                 # half-width (cols per chain)
f32, f16 = mybir.dt.float32, mybir.dt.float16
AFT = mybir.ActivationFunctionType
ALU = mybir.AluOpType
C_HALF_LOG_2PI = 0.9189385332046727
INV_SQRT2 = 0.7071067811865476
EPS = 1e-8

_CACHE = {}


def build_program(sim_compat=False, enable_ph3=True, enable_xproj=True,
                  warm_ldw=0, coalesce=False):
    # sim_compat: CoreSim lacks Erf; substitute Tanh so the rest of the
    # dataflow can be validated locally.
    erf_func = AFT.Tanh if sim_compat else AFT.Erf
    nc = bacc.Bacc("TRN2", target_bir_lowering=False, debug=False,
                   num_devices=NCORES)
    d_xt = {0: nc.dram_tensor("xtA", [2, NSTEP * HWD], f16, kind="ExternalInput"),
            1: nc.dram_tensor("xtB", [2, NSTEP * HWD], f16, kind="ExternalInput")}
    d_lt3 = nc.dram_tensor("lt3", [BL, 2048], f32, kind="ExternalInput")
    d_mw3 = nc.dram_tensor("mw3", [BL, 2048], f32, kind="ExternalInput")
    d_sel3 = nc.dram_tensor("sel3", [BL, 2048], f32, kind="ExternalInput")
    d_whh = nc.dram_tensor("whhT", [128, 128], f16, kind="ExternalInput")
    d_wih = nc.dram_tensor("wihT", [2, 128], f16, kind="ExternalInput")
    d_wlin = nc.dram_tensor("wlinT", [128, 32], f16, kind="ExternalInput")
    d_bv = nc.dram_tensor("bvec", [128, 1], f32, kind="ExternalInput")
    d_nb1 = nc.dram_tensor("nb1vec", [BL, 1], f32, kind="ExternalInput")
    d_b01 = nc.dram_tensor("b01vec", [BL, 1], f32, kind="ExternalInput")
    d_out = nc.dram_tensor("out", [BL, 1], f32, kind="ExternalOutput")

    with tile.TileContext(nc) as tc, ExitStack() as ctx:
        const = ctx.enter_context(tc.tile_pool(name="const", bufs=1))
        ringp = {0: ctx.enter_context(tc.tile_pool(name="ringA", bufs=3)),
                 1: ctx.enter_context(tc.tile_pool(name="ringB", bufs=3))}
        xtp = ctx.enter_context(tc.tile_pool(name="xtp", bufs=3))
        linsb = ctx.enter_context(tc.tile_pool(name="linsb", bufs=2))
        p3sb = ctx.enter_context(tc.tile_pool(name="p3sb", bufs=2))
        ps = {0: ctx.enter_context(tc.tile_pool(name="psA", bufs=3, space="PSUM")),
              1: ctx.enter_context(tc.tile_pool(name="psB", bufs=3, space="PSUM"))}
        ps_l = ctx.enter_context(tc.tile_pool(name="ps_l", bufs=1, space="PSUM"))
        ps_m = ctx.enter_context(tc.tile_pool(name="ps_m", bufs=1, space="PSUM"))

        def load(name, dt_, shape, dtyp):
            t = const.tile(shape, dtyp, tag=name)
            nc.sync.dma_start(t[:], dt_[:])
            return t

        t_lt3 = load("t_lt3", d_lt3, [BL, 2048], f32)
        t_mw3 = load("t_mw3", d_mw3, [BL, 2048], f32)
        t_sel3 = load("t_sel3", d_sel3, [BL, 2048], f32)
        t_whh = load("t_whh", d_whh, [128, 128], f16)
        t_wih = load("t_wih", d_wih, [2, 128], f16)
        t_wlin = load("t_wlin", d_wlin, [128, 32], f16)
        t_bv = load("t_bv", d_bv, [128, 1], f32)
        t_nb1 = load("t_nb1", d_nb1, [BL, 1], f32)
        t_b01 = load("t_b01", d_b01, [BL, 1], f32)

        # dens_acc on 32 partitions (= batch rows): cols 0..15 per
        # (real-window, chain) tail sums; col 16 = sum(mask*(logt'+C));
        # col 17 = mcount*(b0+b1) (lsg reaches the tail without +b1, logt'
        # comes host-side without +b0)
        dens_acc = const.tile([BL, 2 * NRW + 2], f32, tag="dens_acc")
        zsel_acc = const.tile([BL, 2 * NRW], f32, tag="zsel_acc")
        nc.vector.memset(dens_acc[:], 0.0)
        nc.vector.memset(zsel_acc[:], 0.0)
        c_half = const.tile([BL, 1], f32, tag="c_half")
        nc.vector.memset(c_half[:], 0.5)
        mcount = const.tile([BL, 1], f32, tag="mcount")
        nc.vector.tensor_reduce(mcount[:], t_mw3[:], axis=mybir.AxisListType.X,
                                op=ALU.add)
        mwlt = const.tile([BL, 2048], f32, tag="mwlt")
        nc.vector.scalar_tensor_tensor(
            mwlt[:], t_lt3[:], C_HALF_LOG_2PI, t_mw3[:], ALU.add, ALU.mult,
            accum_out=dens_acc[:, 16:17])
        mcb = const.tile([BL, 1], f32, tag="mcb")
        nc.vector.tensor_mul(mcb[:], mcount[:], t_b01[:])
        nc.vector.tensor_copy(dens_acc[:, 17:18], mcb[:])

        xt_tiles = {}      # (window, chain) -> [2, 4096] tile
        ring_tiles = {}    # (window, chain) -> [128, 256*17] tile
        psg_tiles = {}     # (group, chain) -> [128, 512] psum tile
        ls_tiles = {}
        pst_tiles = {}
        pe_fifo = []       # heap of (ready_step, seq, closure)
        fifo_seq = itertools.count()

        def fifo_push(rdy, fn):
            heapq.heappush(pe_fifo, (rdy, next(fifo_seq), fn))

        def emit_xt_dma(w):
            for X in (0, 1):
                t = xtp.tile([2, HWD * 16], f16, tag=f"xt{X}")
                xt_tiles[(w, X)] = t
                nc.sync.dma_start(t[:], d_xt[X][:, HWD * 16 * w:HWD * 16 * (w + 1)])

        def emit_xproj(g):
            # x-projection for steps (2g, 2g+1), both chains: psum group
            # [128, 512] = 2 steps x 256 cols
            w = g // 8
            for X in (0, 1):
                psg = ps[X].tile([128, 512], f32, tag=f"psg{X}")
                psg_tiles[(g, X)] = psg
                if enable_xproj:
                    nc.tensor.matmul(psg[:], t_wih[:],
                                     xt_tiles[(w, X)][:, 512 * (g % 8):512 * (g % 8 + 1)],
                                     start=True, stop=False, skip_group_check=True)

        def enqueue_ph3(w):
            """Phase-3 work for ring window w (real window rw = w-2), spread
            over the following window: one mmlin per step; its 4 transposes 2
            steps later (after the DVE pl->ls copy has surely drained, so
            transposes never stall the PE stream); tails 2 steps after the
            last transpose."""
            rw = w - 2
            j0 = 16 * w + 16
            for X in (0, 1):
                ring = ring_tiles[(w, X)]
                lsT = linsb.tile([BL, 8 * 512], f32, tag=f"lsT{X}")
                ls_tiles[(rw, X)] = lsT
                for g2 in range(8):
                    def mmlin(g2=g2, ring=ring, rw=rw, X=X, lsT=lsT):
                        pl = ps_l.tile([BL, 512], f32, tag="psl")
                        nc.tensor.matmul(pl[:], t_wlin[:],
                                         ring[:, 512 * g2:512 * (g2 + 1)],
                                         start=True, stop=True,
                                         skip_group_check=True)
                        nc.vector.transpose(lsT[:, 512 * g2:512 * (g2 + 1)],
                                            pl[:])
                    rdy = j0 + 2 * g2 + X
                    fifo_push(rdy, mmlin)

                def tail(rw=rw, X=X):
                    emit_ph3_tail(rw, X)
                fifo_push(j0 + 18 + X, tail)

        def emit_ph3_tail(rw, X):
            """mu/sigma -> log-prob contributions for real window rw, chain X.
            lsT[b, 512*g2 + 32*m + e]: e=0 mu, e=1 lsg (cols 2..31 of each
            32-block are replicated copies, ignored)."""
            lsT = ls_tiles.pop((rw, X))
            mu = lsT[:, 0::32]         # [32, 128]; true mu minus b0
            lsg = lsT[:, 1::32]        # true logsig minus b1
            base = (2 * rw + X) * 128
            lt = t_lt3[:, base:base + 128]
            rsig = p3sb.tile([BL, 128], f32, tag="rsig")
            nc.scalar.activation(rsig[:], lsg, AFT.Exp, scale=-1.0,
                                 bias=t_nb1[:])
            zt = p3sb.tile([BL, 128], f32, tag="zt")
            nc.vector.tensor_sub(zt[:], lt, mu)
            z = p3sb.tile([BL, 128], f32, tag="z")
            nc.vector.tensor_mul(z[:], zt[:], rsig[:])
            zsq = p3sb.tile([BL, 128], f32, tag="zsq")
            nc.vector.tensor_mul(zsq[:], z[:], z[:])
            e2 = p3sb.tile([BL, 128], f32, tag="e2")
            nc.vector.scalar_tensor_tensor(e2[:], zsq[:], 0.5, lsg,
                                           ALU.mult, ALU.add)
            m1 = p3sb.tile([BL, 128], f32, tag="m1")
            nc.vector.scalar_tensor_tensor(
                m1[:], e2[:], 1.0, t_mw3[:, base:base + 128],
                ALU.mult, ALU.mult, accum_out=dens_acc[:, 2 * rw + X:2 * rw + X + 1])
            zs = p3sb.tile([BL, 128], f32, tag="zs")
            nc.vector.scalar_tensor_tensor(
                zs[:], z[:], 1.0, t_sel3[:, base:base + 128],
                ALU.mult, ALU.mult, accum_out=zsel_acc[:, 2 * rw + X:2 * rw + X + 1])

        # ---- prologue ----
        emit_xt_dma(0)
        emit_xt_dma(1)
        for X in (0, 1):
            r0 = ringp[X].tile([128, HWD * 17], f16, tag="ring")
            ring_tiles[(0, X)] = r0
            nc.vector.memset(r0[:, 0:HWD], 0.0)      # state_{-1} = 0
        emit_xproj(0)
        emit_xproj(1)

        # ---- main scan ----
        for jj in range(NSTEP):
            w, k = jj // 16, jj % 16
            if k == 0 and w + 2 < NWIN:
                emit_xt_dma(w + 2)
            if jj % 4 == 0:
                for gg in (jj // 2 + 2, jj // 2 + 3):
                    if gg < NSTEP // 2:
                        emit_xproj(gg)
            for X in (0, 1):
                psg = psg_tiles[(jj // 2, X)]
                ring = ring_tiles[(w, X)]
                nc.tensor.matmul(psg[:, 256 * (jj % 2):256 * (jj % 2 + 1)],
                                 t_whh[:], ring[:, HWD * k:HWD * (k + 1)],
                                 start=not enable_xproj, stop=True,
                                 skip_group_check=True)
                nc.scalar.activation(ring[:, HWD * (k + 1):HWD * (k + 2)],
                                     psg[:, 256 * (jj % 2):256 * (jj % 2 + 1)],
                                     AFT.Tanh, bias=t_bv[:])
            if jj % 2 == 1:
                for X in (0, 1):
                    psg_tiles.pop((jj // 2, X), None)
            for _ in range(warm_ldw):
                # dummy stationary reload: keeps the PE HAM activity monitor
                # busy so the array stays at full clock
                nc.tensor.ldweights(t_whh[:])
            if not coalesce or jj % 2 == 1:
                while pe_fifo and pe_fifo[0][0] <= jj:
                    heapq.heappop(pe_fifo)[2]()
            if k == 15:
                if w + 1 < NWIN:
                    for X in (0, 1):
                        rn = ringp[X].tile([128, HWD * 17], f16, tag="ring")
                        ring_tiles[(w + 1, X)] = rn
                        nc.vector.tensor_copy(
                            rn[:, 0:HWD], ring_tiles[(w, X)][:, HWD * 16:HWD * 17])
                    if jj == 31:
                        # chunk 0 restarts exactly from h=0 at its step 0
                        nc.vector.memset(ring_tiles[(2, 0)][:, 0:32], 0.0)
                if w >= 2 and enable_ph3:
                    enqueue_ph3(w)

        # ---- epilogue: drain phase 3, final fold ----
        while pe_fifo:
            heapq.heappop(pe_fifo)[2]()

        zstar = const.tile([BL, 1], f32, tag="zstar")
        dens = const.tile([BL, 1], f32, tag="dens")
        nc.vector.tensor_reduce(zstar[:], zsel_acc[:],
                                axis=mybir.AxisListType.X, op=ALU.add)
        nc.vector.tensor_reduce(dens[:], dens_acc[:],
                                axis=mybir.AxisListType.X, op=ALU.add)
        serf = p3sb.tile([BL, 1], f32, tag="serf")
        nc.scalar.activation(serf[:], zstar[:], erf_func, scale=INV_SQRT2)
        lsv = p3sb.tile([BL, 1], f32, tag="lsv")
        nc.scalar.activation(lsv[:], serf[:], AFT.Ln, bias=c_half[:],
                             scale=-0.5)
        outsb = p3sb.tile([BL, 1], f32, tag="outsb")
        nc.vector.tensor_sub(outsb[:], lsv[:], dens[:])
        nc.sync.dma_start(d_out[:], outsb[:])

    nc.compile()
    return nc


def make_in_maps(times, mask, W_ih, W_hh, b_ih, b_hh, W_lin, b_lin):
    times = np.asarray(times, np.float32)
    mask = np.asarray(mask).astype(bool)
    whhT = np.ascontiguousarray(np.asarray(W_hh, np.float32).T).astype(np.float16)
    wihT = np.ascontiguousarray(np.asarray(W_ih, np.float32).T).astype(np.float16)
    wlin2 = np.ascontiguousarray(np.asarray(W_lin, np.float32).T)   # [128, 2]
    wlinT = np.tile(wlin2, (1, 16)).astype(np.float16)              # [128, 32]
    bvec = (np.asarray(b_ih, np.float32) + np.asarray(b_hh, np.float32)).reshape(H, 1)
    b0, b1 = float(b_lin[0]), float(b_lin[1])
    nb1vec = np.full((BL, 1), -b1, np.float32)
    b01vec = np.full((BL, 1), b0 + b1, np.float32)

    chunks = np.arange(P)
    sg = CH * chunks[:, None] + np.arange(NSTEP)[None, :] - WU     # [16,160]
    valid = sg >= 0
    sgc = np.clip(sg, 0, S - 1)

    c3, j3, b3 = np.meshgrid(np.arange(P), np.arange(CH), np.arange(BL),
                             indexing="ij")
    cc3, ch3 = c3 % 8, c3 // 8
    p_idx = b3
    col_idx = ((2 * (j3 // 16) + ch3) * 128 + 16 * ((j3 % 16) // 2)
               + 8 * (j3 % 2) + cc3)
    s3 = CH * c3 + j3

    in_maps = []
    for c in range(NCORES):
        tc_ = times[BL * c:BL * (c + 1)]                # [32, 2048]
        mc = mask[BL * c:BL * (c + 1)]
        t = np.maximum(tc_, EPS)
        lt = np.log(t)

        tv = np.where(valid[None], t[:, sgc], 1.0)      # [32, 16, 160]
        ltv = np.where(valid[None], lt[:, sgc], 0.0)

        def xt_for(chain):
            sel = slice(8 * chain, 8 * chain + 8)
            a = np.stack([tv[:, sel], ltv[:, sel]])     # [2, 32b, 8cc, 160jj]
            return np.ascontiguousarray(
                a.transpose(0, 3, 2, 1).reshape(2, NSTEP * HWD)).astype(np.float16)
        xtA, xtB = xt_for(0), xt_for(1)

        lt3 = np.zeros((BL, 2048), np.float32)
        mw3 = np.zeros((BL, 2048), np.float32)
        sel3 = np.zeros((BL, 2048), np.float32)
        mw_full = np.concatenate([mc[:, 1:].astype(np.float32),
                                  np.zeros((BL, 1), np.float32)], axis=1)
        sstar = mc.sum(1).astype(np.int64) - 1
        selA = np.zeros((BL, S), np.float32)
        selA[np.arange(BL), sstar] = 1.0
        lt3[p_idx, col_idx] = lt[b3, s3] - b0      # b_lin[0] folded into logt
        mw3[p_idx, col_idx] = mw_full[b3, s3]
        sel3[p_idx, col_idx] = selA[b3, s3]

        in_maps.append({
            "xtA": xtA, "xtB": xtB,
            "lt3": lt3, "mw3": mw3, "sel3": sel3,
            "whhT": whhT, "wihT": wihT, "wlinT": wlinT,
            "bvec": bvec, "nb1vec": nb1vec, "b01vec": b01vec,
        })
    return in_maps


def make_runner(nc, n_cores=NCORES):
    """Build a reusable jitted SPMD callable (compiles once)."""
    import jax
    from jax.sharding import Mesh, PartitionSpec
    from jax.experimental.shard_map import shard_map

    bass2jax.install_neuronx_cc_hook()
    partition_name = nc.partition_id_tensor.name if nc.partition_id_tensor else None
    in_names, out_names, out_avals, zero_outs = [], [], [], []
    for alloc in nc.m.functions[0].allocations:
        if not isinstance(alloc, mybir.MemoryLocationSet):
            continue
        name = alloc.memorylocations[0].name
        if alloc.kind == "ExternalInput":
            if name != partition_name:
                in_names.append(name)
        elif alloc.kind == "ExternalOutput":
            out_names.append(name)
            shape = tuple(alloc.tensor_shape)
            dtype = mybir.dt.np(alloc.dtype)
            out_avals.append(jax.core.ShapedArray(shape, dtype))
            zero_outs.append(np.zeros(shape, dtype))
    n_params = len(in_names)
    n_outs = len(out_avals)
    in_names_all = list(in_names) + out_names
    if partition_name is not None:
        in_names_all.append(partition_name)
    donate = tuple(range(n_params, n_params + n_outs))

    def _body(*args):
        operands = list(args)
        if partition_name is not None:
            operands.append(bass2jax.partition_id_tensor())
        outs = bass2jax._bass_exec_p.bind(
            *operands,
            out_avals=tuple(out_avals),
            in_names=tuple(in_names_all),
            out_names=tuple(out_names),
            lowering_input_output_aliases=(),
            sim_require_finite=True,
            sim_require_nnan=True,
            nc=nc,
        )
        return tuple(outs)

    devices = jax.devices()[:n_cores]
    mesh = Mesh(np.asarray(devices), ("core",))
    in_specs = (PartitionSpec("core"),) * (n_params + n_outs)
    out_specs = (PartitionSpec("core"),) * len(out_names)
    sharded = jax.jit(
        shard_map(_body, mesh=mesh, in_specs=in_specs, out_specs=out_specs,
                  check_rep=False),
        donate_argnums=donate, keep_unused=True)

    def run(in_maps):
        import jax
        per_core = [[np.asarray(m[name]) for name in in_names] for m in in_maps]
        concat_in = [np.concatenate([per_core[c][i] for c in range(n_cores)], axis=0)
                     for i in range(n_params)]
        concat_zeros = [np.zeros((n_cores * z.shape[0], *z.shape[1:]), z.dtype)
                        for z in zero_outs]
        out_arrs = sharded(*concat_in, *concat_zeros)
        jax.block_until_ready(out_arrs)
        return [
            {name: np.asarray(out_arrs[i]).reshape(n_cores, *out_avals[i].shape)[c]
             for i, name in enumerate(out_names)}
            for c in range(n_cores)
        ]
    return run


def _get_runner():
    if "runner" not in _CACHE:
        nc = build_program()
        _CACHE["nc"] = nc
        _CACHE["runner"] = make_runner(nc)
    return _CACHE["runner"]


def kernel(times, mask, W_ih, W_hh, b_ih, b_hh, W_lin, b_lin):
    in_maps = make_in_maps(times, mask, W_ih, W_hh, b_ih, b_hh, W_lin, b_lin)
    runner = _get_runner()
    outs = runner(in_maps)
    return np.concatenate([outs[c]["out"][:, 0] for c in range(NCORES)]).astype(np.float32)


# revision 33
# speedup vs baseline: 2.9271x; 2.9271x over previous
"""NeuralTPP log-likelihood kernel for 8x Trainium2 NeuronCores.

Reference computation (per batch row b):
  t = max(times, 1e-8); logt = log(t); x = [t, logt]
  h_s = tanh(W_ih x_s + b_ih + b_hh + W_hh h_{s-1}),  h_{-1} = 0   (S=2048)
  [mu_s, logsig_s] = W_lin h_{s-1} + b_lin
  z_s = (logt_s - mu_s) / exp(logsig_s)
  log_density = sum_{s<=S-2} mask[s+1] * (-logt_s - logsig_s - C - z_s^2/2)
  last = log(0.5 - 0.5*erf(z_{s*}/sqrt(2))),  s* = sum(mask) - 1
  out  = log_density + last

Strategy: data parallel over batch (32 rows/core) PLUS sequence-parallel
within each core. The tanh RNN is strongly contractive (cold restart
converges to float noise in <48 steps), so S=2048 splits into 16 chunks of
128 steps, each warmed up from h=0 over the preceding 32 steps. All chunks
advance in lockstep: the 2048-step serial scan becomes 160 steps of
512-wide ops (col = 32*chunk + b), run as two 256-wide half-chains (A =
chunks 0-7, B = 8-15) so PE-matmul and ACT-tanh of the two halves overlap.
x-projections are pre-accumulated into PSUM 2 steps per bank; the output
side (mu/sigma/log-prob) runs on ring windows of 16 steps, one window
behind the scan, via a PE-op fifo drained between chain matmuls.
"""
import heapq
import itertools
import numpy as np
from contextlib import ExitStack

import concourse.bacc as bacc
import concourse.bass as bass
import concourse.tile as tile
import concourse.mybir as mybir
from concourse import bass2jax

B, S, H = 256, 2048, 128
NCORES = 8
BL = B // NCORES          # 32 batch rows per core
P = 16                    # sequence chunks
CH = S // P               # 128 steps per chunk
WU = 32                   # warmup steps
NSTEP = WU + CH           # 160 serial steps
NWIN = NSTEP // 16        # 10 ring windows (2 warmup + 8 real)
NRW = CH // 16            # 8 real windows
HWD = 256                 # half-width (cols per chain)
f32, f16 = mybir.dt.float32, mybir.dt.float16
AFT = mybir.ActivationFunctionType
ALU = mybir.AluOpType
C_HALF_LOG_2PI = 0.9189385332046727
INV_SQRT2 = 0.7071067811865476
EPS = 1e-8

_CACHE = {}


def build_program(sim_compat=False, enable_ph3=True, enable_xproj=True,
                  anti_phase=False):
    # sim_compat: CoreSim lacks Erf; substitute Tanh so the rest of the
    # dataflow can be validated locally.
    erf_func = AFT.Tanh if sim_compat else AFT.Erf
    nc = bacc.Bacc("TRN2", target_bir_lowering=False, debug=False,
                   num_devices=NCORES)
    d_xt = {0: nc.dram_tensor("xtA", [2, NSTEP * HWD], f16, kind="ExternalInput"),
            1: nc.dram_tensor("xtB", [2, NSTEP * HWD], f16, kind="ExternalInput")}
    d_lt3 = nc.dram_tensor("lt3", [BL, 2048], f32, kind="ExternalInput")
    d_mw3 = nc.dram_tensor("mw3", [BL, 2048], f32, kind="ExternalInput")
    d_sel3 = nc.dram_tensor("sel3", [BL, 2048], f32, kind="ExternalInput")
    d_whh = nc.dram_tensor("whhT", [128, 128], f16, kind="ExternalInput")
    d_wih = nc.dram_tensor("wihT", [2, 128], f16, kind="ExternalInput")
    d_wlin = nc.dram_tensor("wlinT", [128, 32], f16, kind="ExternalInput")
    d_bv = nc.dram_tensor("bvec", [128, 1], f32, kind="ExternalInput")
    d_nb1 = nc.dram_tensor("nb1vec", [BL, 1], f32, kind="ExternalInput")
    d_b01 = nc.dram_tensor("b01vec", [BL, 1], f32, kind="ExternalInput")
    d_out = nc.dram_tensor("out", [BL, 1], f32, kind="ExternalOutput")

    with tile.TileContext(nc) as tc, ExitStack() as ctx:
        const = ctx.enter_context(tc.tile_pool(name="const", bufs=1))
        ringp = {0: ctx.enter_context(tc.tile_pool(name="ringA", bufs=3)),
                 1: ctx.enter_context(tc.tile_pool(name="ringB", bufs=3))}
        xtp = ctx.enter_context(tc.tile_pool(name="xtp", bufs=3))
        linsb = ctx.enter_context(tc.tile_pool(name="linsb", bufs=2))
        p3sb = ctx.enter_context(tc.tile_pool(name="p3sb", bufs=2))
        ps = {0: ctx.enter_context(tc.tile_pool(name="psA", bufs=3, space="PSUM")),
              1: ctx.enter_context(tc.tile_pool(name="psB", bufs=3, space="PSUM"))}
        ps_l = ctx.enter_context(tc.tile_pool(name="ps_l", bufs=2, space="PSUM"))

        def load(name, dt_, shape, dtyp):
            t = const.tile(shape, dtyp, tag=name)
            nc.sync.dma_start(t[:], dt_[:])
            return t

        t_lt3 = load("t_lt3", d_lt3, [BL, 2048], f32)
        t_mw3 = load("t_mw3", d_mw3, [BL, 2048], f32)
        t_sel3 = load("t_sel3", d_sel3, [BL, 2048], f32)
        t_whh = load("t_whh", d_whh, [128, 128], f16)
        t_wih = load("t_wih", d_wih, [2, 128], f16)
        t_wlin = load("t_wlin", d_wlin, [128, 32], f16)
        t_bv = load("t_bv", d_bv, [128, 1], f32)
        t_nb1 = load("t_nb1", d_nb1, [BL, 1], f32)
        t_b01 = load("t_b01", d_b01, [BL, 1], f32)

        # dens_acc on 32 partitions (= batch rows): cols 0..15 per
        # (real-window, chain) tail sums; col 16 = sum(mask*(logt'+C));
        # col 17 = mcount*(b0+b1) (lsg reaches the tail without +b1, logt'
        # comes host-side without +b0)
        dens_acc = const.tile([BL, 2 * NRW + 2], f32, tag="dens_acc")
        zsel_acc = const.tile([BL, 2 * NRW], f32, tag="zsel_acc")
        nc.vector.memset(dens_acc[:], 0.0)
        nc.vector.memset(zsel_acc[:], 0.0)
        c_half = const.tile([BL, 1], f32, tag="c_half")
        nc.vector.memset(c_half[:], 0.5)
        mcount = const.tile([BL, 1], f32, tag="mcount")
        nc.vector.tensor_reduce(mcount[:], t_mw3[:], axis=mybir.AxisListType.X,
                                op=ALU.add)
        mwlt = const.tile([BL, 2048], f32, tag="mwlt")
        nc.vector.scalar_tensor_tensor(
            mwlt[:], t_lt3[:], C_HALF_LOG_2PI, t_mw3[:], ALU.add, ALU.mult,
            accum_out=dens_acc[:, 16:17])
        mcb = const.tile([BL, 1], f32, tag="mcb")
        nc.vector.tensor_mul(mcb[:], mcount[:], t_b01[:])
        nc.vector.tensor_copy(dens_acc[:, 17:18], mcb[:])

        xt_tiles = {}      # (window, chain) -> [2, 4096] tile
        ring_tiles = {}    # (window, chain) -> [128, 256*17] tile
        psg_tiles = {}     # (group, chain) -> [128, 512] psum tile
        ls_tiles = {}
        pst_tiles = {}
        pe_fifo = []       # heap of (ready_step, seq, closure)
        fifo_seq = itertools.count()

        def fifo_push(rdy, fn):
            heapq.heappush(pe_fifo, (rdy, next(fifo_seq), fn))

        def emit_xt_dma(w):
            for X in (0, 1):
                t = xtp.tile([2, HWD * 16], f16, tag=f"xt{X}")
                xt_tiles[(w, X)] = t
                nc.sync.dma_start(t[:], d_xt[X][:, HWD * 16 * w:HWD * 16 * (w + 1)])

        def emit_xproj(g):
            # x-projection for steps (2g, 2g+1), both chains: psum group
            # [128, 512] = 2 steps x 256 cols
            w = g // 8
            for X in (0, 1):
                psg = ps[X].tile([128, 512], f32, tag=f"psg{X}")
                psg_tiles[(g, X)] = psg
                if enable_xproj:
                    nc.tensor.matmul(psg[:], t_wih[:],
                                     xt_tiles[(w, X)][:, 512 * (g % 8):512 * (g % 8 + 1)],
                                     start=True, stop=False, skip_group_check=True)

        def enqueue_ph3_last(w):
            """Last window: per-g2 readies so the drain overlaps the chain's
            final steps instead of serializing after it."""
            rw = w - 2
            for X in (0, 1):
                ring = ring_tiles[(w, X)]
                lsT = linsb.tile([BL, 8 * 512], f32, tag=f"lsT{X}")
                ls_tiles[(rw, X)] = lsT
                for g2 in range(8):
                    def mmlin(g2=g2, ring=ring, rw=rw, X=X, lsT=lsT):
                        pl = ps_l.tile([BL, 512], f32, tag="psl")
                        nc.tensor.matmul(pl[:], t_wlin[:],
                                         ring[:, 512 * g2:512 * (g2 + 1)],
                                         start=True, stop=True,
                                         skip_group_check=True)
                        nc.vector.transpose(lsT[:, 512 * g2:512 * (g2 + 1)],
                                            pl[:])
                    fifo_push(16 * w + 2 * g2 + 2 + X, mmlin)

                def tail(rw=rw, X=X):
                    emit_ph3_tail(rw, X)
                fifo_push(16 * w + 17 + X, tail)

        def enqueue_ph3(w):
            """Phase-3 work for ring window w (real window rw = w-2), spread
            over the following window: one mmlin per step; its 4 transposes 2
            steps later (after the DVE pl->ls copy has surely drained, so
            transposes never stall the PE stream); tails 2 steps after the
            last transpose."""
            rw = w - 2
            j0 = 16 * w + 16
            for X in (0, 1):
                ring = ring_tiles[(w, X)]
                lsT = linsb.tile([BL, 8 * 512], f32, tag=f"lsT{X}")
                ls_tiles[(rw, X)] = lsT
                for g2 in range(8):
                    def mmlin(g2=g2, ring=ring, rw=rw, X=X, lsT=lsT):
                        pl = ps_l.tile([BL, 512], f32, tag="psl")
                        nc.tensor.matmul(pl[:], t_wlin[:],
                                         ring[:, 512 * g2:512 * (g2 + 1)],
                                         start=True, stop=True,
                                         skip_group_check=True)
                        nc.vector.transpose(lsT[:, 512 * g2:512 * (g2 + 1)],
                                            pl[:])
                    rdy = j0 + 2 * g2 + X
                    fifo_push(rdy, mmlin)

                def tail(rw=rw, X=X):
                    emit_ph3_tail(rw, X)
                fifo_push(j0 + 18 + X, tail)

        def emit_ph3_tail(rw, X):
            """mu/sigma -> log-prob contributions for real window rw, chain X.
            lsT[b, 512*g2 + 32*m + e]: e=0 mu, e=1 lsg (cols 2..31 of each
            32-block are replicated copies, ignored)."""
            lsT = ls_tiles.pop((rw, X))
            mu = lsT[:, 0::32]         # [32, 128]; true mu minus b0
            lsg = lsT[:, 1::32]        # true logsig minus b1
            base = (2 * rw + X) * 128
            lt = t_lt3[:, base:base + 128]
            rsig = p3sb.tile([BL, 128], f32, tag="rsig")
            nc.scalar.activation(rsig[:], lsg, AFT.Exp, scale=-1.0,
                                 bias=t_nb1[:])
            zt = p3sb.tile([BL, 128], f32, tag="zt")
            nc.vector.tensor_sub(zt[:], lt, mu)
            z = p3sb.tile([BL, 128], f32, tag="z")
            nc.vector.tensor_mul(z[:], zt[:], rsig[:])
            zsq = p3sb.tile([BL, 128], f32, tag="zsq")
            nc.vector.tensor_mul(zsq[:], z[:], z[:])
            e2 = p3sb.tile([BL, 128], f32, tag="e2")
            nc.vector.scalar_tensor_tensor(e2[:], zsq[:], 0.5, lsg,
                                           ALU.mult, ALU.add)
            m1 = p3sb.tile([BL, 128], f32, tag="m1")
            nc.vector.scalar_tensor_tensor(
                m1[:], e2[:], 1.0, t_mw3[:, base:base + 128],
                ALU.mult, ALU.mult, accum_out=dens_acc[:, 2 * rw + X:2 * rw + X + 1])
            zs = p3sb.tile([BL, 128], f32, tag="zs")
            nc.vector.scalar_tensor_tensor(
                zs[:], z[:], 1.0, t_sel3[:, base:base + 128],
                ALU.mult, ALU.mult, accum_out=zsel_acc[:, 2 * rw + X:2 * rw + X + 1])

        # ---- prologue ----
        emit_xt_dma(0)
        emit_xt_dma(1)
        for X in (0, 1):
            r0 = ringp[X].tile([128, HWD * 17], f16, tag="ring")
            ring_tiles[(0, X)] = r0
            nc.vector.memset(r0[:, 0:HWD], 0.0)      # state_{-1} = 0
        emit_xproj(0)
        emit_xproj(1)

        # ---- main scan ----
        for jj in range(NSTEP):
            w, k = jj // 16, jj % 16
            if k == 0 and w + 2 < NWIN:
                emit_xt_dma(w + 2)
            if jj % 4 == 0:
                for gg in (jj // 2 + 2, jj // 2 + 3):
                    if gg < NSTEP // 2:
                        emit_xproj(gg)
            def chain_step(X):
                psg = psg_tiles[(jj // 2, X)]
                ring = ring_tiles[(w, X)]
                nc.tensor.matmul(psg[:, 256 * (jj % 2):256 * (jj % 2 + 1)],
                                 t_whh[:], ring[:, HWD * k:HWD * (k + 1)],
                                 start=not enable_xproj, stop=True,
                                 skip_group_check=True)
                nc.scalar.activation(ring[:, HWD * (k + 1):HWD * (k + 2)],
                                     psg[:, 256 * (jj % 2):256 * (jj % 2 + 1)],
                                     AFT.Tanh, bias=t_bv[:])

            def pops(budget):
                n = 0
                while pe_fifo and pe_fifo[0][0] <= jj and n < budget:
                    heapq.heappop(pe_fifo)[2]()
                    n += 1

            if anti_phase:
                # emit the two half-chains' ops half a step apart so the
                # static scheduler staggers them (anti-phase) instead of
                # running both matmuls back-to-back
                chain_step(0)
                pops(3)
                chain_step(1)
                pops(99)
            else:
                chain_step(0)
                chain_step(1)
                pops(99)
            if jj % 2 == 1:
                for X in (0, 1):
                    psg_tiles.pop((jj // 2, X), None)
            if k == 15:
                if w + 1 < NWIN:
                    for X in (0, 1):
                        rn = ringp[X].tile([128, HWD * 17], f16, tag="ring")
                        ring_tiles[(w + 1, X)] = rn
                        nc.vector.tensor_copy(
                            rn[:, 0:HWD], ring_tiles[(w, X)][:, HWD * 16:HWD * 17])
                    if jj == 31:
                        # chunk 0 restarts exactly from h=0 at its step 0
                        nc.vector.memset(ring_tiles[(2, 0)][:, 0:32], 0.0)
                    if w + 1 == NWIN - 1 and enable_ph3:
                        # last window: enqueue now with per-g2 readies so its
                        # phase-3 overlaps the chain's final steps
                        enqueue_ph3_last(w + 1)
                if w >= 2 and enable_ph3 and w != NWIN - 1:
                    enqueue_ph3(w)

        # ---- epilogue: drain phase 3, final fold ----
        while pe_fifo:
            heapq.heappop(pe_fifo)[2]()

        zstar = const.tile([BL, 1], f32, tag="zstar")
        dens = const.tile([BL, 1], f32, tag="dens")
        nc.vector.tensor_reduce(zstar[:], zsel_acc[:],
                                axis=mybir.AxisListType.X, op=ALU.add)
        nc.vector.tensor_reduce(dens[:], dens_acc[:],
                                axis=mybir.AxisListType.X, op=ALU.add)
        serf = p3sb.tile([BL, 1], f32, tag="serf")
        nc.scalar.activation(serf[:], zstar[:], erf_func, scale=INV_SQRT2)
        lsv = p3sb.tile([BL, 1], f32, tag="lsv")
        nc.scalar.activation(lsv[:], serf[:], AFT.Ln, bias=c_half[:],
                             scale=-0.5)
        outsb = p3sb.tile([BL, 1], f32, tag="outsb")
        nc.vector.tensor_sub(outsb[:], lsv[:], dens[:])
        nc.sync.dma_start(d_out[:], outsb[:])

    nc.compile()
    return nc


def make_in_maps(times, mask, W_ih, W_hh, b_ih, b_hh, W_lin, b_lin):
    times = np.asarray(times, np.float32)
    mask = np.asarray(mask).astype(bool)
    whhT = np.ascontiguousarray(np.asarray(W_hh, np.float32).T).astype(np.float16)
    wihT = np.ascontiguousarray(np.asarray(W_ih, np.float32).T).astype(np.float16)
    wlin2 = np.ascontiguousarray(np.asarray(W_lin, np.float32).T)   # [128, 2]
    wlinT = np.tile(wlin2, (1, 16)).astype(np.float16)              # [128, 32]
    bvec = (np.asarray(b_ih, np.float32) + np.asarray(b_hh, np.float32)).reshape(H, 1)
    b0, b1 = float(b_lin[0]), float(b_lin[1])
    nb1vec = np.full((BL, 1), -b1, np.float32)
    b01vec = np.full((BL, 1), b0 + b1, np.float32)

    chunks = np.arange(P)
    sg = CH * chunks[:, None] + np.arange(NSTEP)[None, :] - WU     # [16,160]
    valid = sg >= 0
    sgc = np.clip(sg, 0, S - 1)

    c3, j3, b3 = np.meshgrid(np.arange(P), np.arange(CH), np.arange(BL),
                             indexing="ij")
    cc3, ch3 = c3 % 8, c3 // 8
    p_idx = b3
    col_idx = ((2 * (j3 // 16) + ch3) * 128 + 16 * ((j3 % 16) // 2)
               + 8 * (j3 % 2) + cc3)
    s3 = CH * c3 + j3

    in_maps = []
    for c in range(NCORES):
        tc_ = times[BL * c:BL * (c + 1)]                # [32, 2048]
        mc = mask[BL * c:BL * (c + 1)]
        t = np.maximum(tc_, EPS)
        lt = np.log(t)

        tv = np.where(valid[None], t[:, sgc], 1.0)      # [32, 16, 160]
        ltv = np.where(valid[None], lt[:, sgc], 0.0)

        def xt_for(chain):
            sel = slice(8 * chain, 8 * chain + 8)
            a = np.stack([tv[:, sel], ltv[:, sel]])     # [2, 32b, 8cc, 160jj]
            return np.ascontiguousarray(
                a.transpose(0, 3, 2, 1).reshape(2, NSTEP * HWD)).astype(np.float16)
        xtA, xtB = xt_for(0), xt_for(1)

        lt3 = np.zeros((BL, 2048), np.float32)
        mw3 = np.zeros((BL, 2048), np.float32)
        sel3 = np.zeros((BL, 2048), np.float32)
        mw_full = np.concatenate([mc[:, 1:].astype(np.float32),
                                  np.zeros((BL, 1), np.float32)], axis=1)
        sstar = mc.sum(1).astype(np.int64) - 1
        selA = np.zeros((BL, S), np.float32)
        selA[np.arange(BL), sstar] = 1.0
        lt3[p_idx, col_idx] = lt[b3, s3] - b0      # b_lin[0] folded into logt
        mw3[p_idx, col_idx] = mw_full[b3, s3]
        sel3[p_idx, col_idx] = selA[b3, s3]

        in_maps.append({
            "xtA": xtA, "xtB": xtB,
            "lt3": lt3, "mw3": mw3, "sel3": sel3,
            "whhT": whhT, "wihT": wihT, "wlinT": wlinT,
            "bvec": bvec, "nb1vec": nb1vec, "b01vec": b01vec,
        })
    return in_maps


def make_runner(nc, n_cores=NCORES):
    """Build a reusable jitted SPMD callable (compiles once)."""
    import jax
    from jax.sharding import Mesh, PartitionSpec
    from jax.experimental.shard_map import shard_map

    bass2jax.install_neuronx_cc_hook()
    partition_name = nc.partition_id_tensor.name if nc.partition_id_tensor else None
    in_names, out_names, out_avals, zero_outs = [], [], [], []
    for alloc in nc.m.functions[0].allocations:
        if not isinstance(alloc, mybir.MemoryLocationSet):
            continue
        name = alloc.memorylocations[0].name
        if alloc.kind == "ExternalInput":
            if name != partition_name:
                in_names.append(name)
        elif alloc.kind == "ExternalOutput":
            out_names.append(name)
            shape = tuple(alloc.tensor_shape)
            dtype = mybir.dt.np(alloc.dtype)
            out_avals.append(jax.core.ShapedArray(shape, dtype))
            zero_outs.append(np.zeros(shape, dtype))
    n_params = len(in_names)
    n_outs = len(out_avals)
    in_names_all = list(in_names) + out_names
    if partition_name is not None:
        in_names_all.append(partition_name)
    donate = tuple(range(n_params, n_params + n_outs))

    def _body(*args):
        operands = list(args)
        if partition_name is not None:
            operands.append(bass2jax.partition_id_tensor())
        outs = bass2jax._bass_exec_p.bind(
            *operands,
            out_avals=tuple(out_avals),
            in_names=tuple(in_names_all),
            out_names=tuple(out_names),
            lowering_input_output_aliases=(),
            sim_require_finite=True,
            sim_require_nnan=True,
            nc=nc,
        )
        return tuple(outs)

    devices = jax.devices()[:n_cores]
    mesh = Mesh(np.asarray(devices), ("core",))
    in_specs = (PartitionSpec("core"),) * (n_params + n_outs)
    out_specs = (PartitionSpec("core"),) * len(out_names)
    sharded = jax.jit(
        shard_map(_body, mesh=mesh, in_specs=in_specs, out_specs=out_specs,
                  check_rep=False),
        donate_argnums=donate, keep_unused=True)

    def run(in_maps):
        import jax
        per_core = [[np.asarray(m[name]) for name in in_names] for m in in_maps]
        concat_in = [np.concatenate([per_core[c][i] for c in range(n_cores)], axis=0)
                     for i in range(n_params)]
        concat_zeros = [np.zeros((n_cores * z.shape[0], *z.shape[1:]), z.dtype)
                        for z in zero_outs]
        out_arrs = sharded(*concat_in, *concat_zeros)
        jax.block_until_ready(out_arrs)
        return [
            {name: np.asarray(out_arrs[i]).reshape(n_cores, *out_avals[i].shape)[c]
             for i, name in enumerate(out_names)}
            for c in range(n_cores)
        ]
    return run


def _get_runner():
    if "runner" not in _CACHE:
        nc = build_program()
        _CACHE["nc"] = nc
        _CACHE["runner"] = make_runner(nc)
    return _CACHE["runner"]


def kernel(times, mask, W_ih, W_hh, b_ih, b_hh, W_lin, b_lin):
    in_maps = make_in_maps(times, mask, W_ih, W_hh, b_ih, b_hh, W_lin, b_lin)
    runner = _get_runner()
    outs = runner(in_maps)
    return np.concatenate([outs[c]["out"][:, 0] for c in range(NCORES)]).astype(np.float32)
